# revision 10
# baseline (speedup 1.0000x reference)
"""Trainium2 Bass kernel for nn_GBM_68539088109972.

Encoder (32768->1024) -> Mamba block (d_inner=2048, selective scan L=256) ->
Decoder (1024->32768), B=2, L=256, distributed over 8 NeuronCores:
  - encoder : K-parallel over FLAT (AllReduce of lat, 2MB f32)
  - mamba   : tensor-parallel over d_inner (AllReduce of dbc 192KB + out_proj 2MB)
  - decoder : M-parallel over FLAT rows (no comm; host gathers)
Activations live in channel-partition layout [channels, B*L] throughout.
The selective scan uses the native DVE tensor_tensor_scan instruction with
state segments laid out n-major in the free dim; a zero in dA at each
segment head resets the recurrence.
"""

import sys

sys.path.insert(0, "/opt/trn_rl_repo")

import numpy as np
import ml_dtypes

import concourse.bass as bass
import concourse.tile as tile
from concourse import bacc, mybir
from concourse import bass_utils

BF16 = ml_dtypes.bfloat16
N_CORES = 8
B, L = 2, 256
BL = B * L                      # 512
H, W = 256, 128
FLAT = H * W                    # 32768
D_MODEL = 1024
D_INNER = 2048
D_STATE = 16
D_CONV = 4
DT_RANK = 64
EPS = 1e-5

P = 128
FLAT_SH = FLAT // N_CORES       # 4096
DI_SH = D_INNER // N_CORES      # 256
KE = FLAT_SH // P               # 32 encoder K tiles
MD = D_MODEL // P               # 8
TI = DI_SH // P                 # 2 d_inner tiles per core
MF = FLAT_SH // P               # 32 decoder M tiles

f32 = mybir.dt.float32
bf16 = mybir.dt.bfloat16


def _body(tc, io):
    nc = tc.nc
    Act = mybir.ActivationFunctionType
    Alu = mybir.AluOpType
    RG = [list(range(N_CORES))]

    from contextlib import ExitStack
    ctx = ExitStack()
    const = ctx.enter_context(tc.tile_pool(name="const", bufs=1))
    wstream = ctx.enter_context(tc.tile_pool(name="wstream", bufs=4))
    xstream = ctx.enter_context(tc.tile_pool(name="xstream", bufs=4))
    latp = ctx.enter_context(tc.tile_pool(name="latp", bufs=1))
    scanp = ctx.enter_context(tc.tile_pool(name="scanp", bufs=1))
    big = ctx.enter_context(tc.tile_pool(name="big", bufs=1))
    outp = ctx.enter_context(tc.tile_pool(name="outp", bufs=3))
    psum = ctx.enter_context(tc.tile_pool(name="psum", bufs=8, space="PSUM"))
    dram = ctx.enter_context(tc.tile_pool(name="dram", bufs=1, space="DRAM"))

    # ---------------- encoder: lat_partial = enc_wT_sh.T @ xT_sh ----------------
    enc_ps = [psum.tile([P, BL], f32, name=f"encps{m}", tag="mmps") for m in range(MD)]
    for k in range(KE):
        xk = xstream.tile([P, BL], bf16, tag="xk")
        nc.sync.dma_start(xk[:], io["xT"][k * P:(k + 1) * P, :])
        wk = wstream.tile([P, D_MODEL], bf16, tag="wk")
        nc.sync.dma_start(wk[:], io["enc_wT"][k * P:(k + 1) * P, :])
        for m in range(MD):
            nc.tensor.matmul(
                enc_ps[m][:], lhsT=wk[:, m * P:(m + 1) * P], rhs=xk[:],
                start=(k == 0), stop=(k == KE - 1),
            )

    ar1_in = dram.tile([D_MODEL, BL], f32, name="ar1_in")
    ar1_out = dram.tile([D_MODEL, BL], f32, name="ar1_out", addr_space="Shared")
    for m in range(MD):
        lp = latp.tile([P, BL], f32, name=f"latpar{m}", tag="xnhm")
        nc.vector.tensor_copy(lp[:], enc_ps[m][:])
        nc.sync.dma_start(ar1_in[m * P:(m + 1) * P, :], lp[:])
    nc.gpsimd.collective_compute(
        "AllReduce", Alu.add, replica_groups=RG,
        ins=[ar1_in.opt()], outs=[ar1_out.opt()],
    )

    # ---------------- lat reload + enc_b + rmsnorm ----------------
    enc_b_sb = const.tile([P, MD], f32, name="encb")
    nc.sync.dma_start(enc_b_sb[:], io["enc_b"].rearrange("(o p) -> p o", p=P))
    ones_sb = const.tile([P, 1], bf16, name="ones")
    nc.vector.memset(ones_sb[:], 1.0)

    ss_ps = psum.tile([1, BL], f32, name="ssps", tag="mmps")
    lat_sb = []
    for m in range(MD):
        lt = latp.tile([P, BL], f32, name=f"lat{m}")
        nc.sync.dma_start(lt[:], ar1_out[m * P:(m + 1) * P, :])
        nc.vector.tensor_scalar_add(lt[:], lt[:], enc_b_sb[:, m:m + 1])
        lat_sb.append(lt)
        sq = xstream.tile([P, BL], bf16, tag="sq")
        nc.scalar.activation(sq[:], lt[:], Act.Square)
        nc.tensor.matmul(ss_ps[:], lhsT=ones_sb[:], rhs=sq[:],
                         start=(m == 0), stop=(m == MD - 1))

    eps_sb = const.tile([1, 1], f32, name="eps")
    nc.vector.memset(eps_sb[:], EPS)
    rms = scanp.tile([1, BL], f32, name="rms")
    nc.scalar.activation(rms[:], ss_ps[:], Act.Sqrt, bias=eps_sb[:], scale=1.0 / D_MODEL)
    rstd = scanp.tile([1, BL], f32, name="rstd")
    nc.vector.reciprocal(rstd[:], rms[:])
    rstd_dr = dram.tile([1, BL], f32, name="rstd_dr")
    nc.sync.dma_start(rstd_dr[:, :], rstd[:])
    rstd_bc = scanp.tile([P, BL], f32, name="rstdbc")
    nc.sync.dma_start(rstd_bc[:], rstd_dr[0:1, :].to_broadcast((P, BL)))

    xn_bf = []
    for m in range(MD):
        xb = latp.tile([P, BL], bf16, name=f"xn{m}", tag="xnhm")
        nc.vector.tensor_tensor(xb[:], lat_sb[m][:], rstd_bc[:], Alu.mult)
        xn_bf.append(xb)

    # ---------------- in_proj: u,z = (W' @ xn) ----------------
    uz_ps = [psum.tile([P, BL], f32, name=f"uzps{mi}", tag="mmps") for mi in range(4)]
    for k in range(MD):
        wik = wstream.tile([P, 2 * DI_SH], bf16, tag="wk")
        nc.sync.dma_start(wik[:, :2 * DI_SH], io["in_projT"][k * P:(k + 1) * P, :])
        for mi in range(4):
            nc.tensor.matmul(
                uz_ps[mi][:], lhsT=wik[:, mi * P:(mi + 1) * P], rhs=xn_bf[k][:],
                start=(k == 0), stop=(k == MD - 1),
            )

    # ---------------- conv (causal, depthwise) + silu; silu(z) ----------------
    conv_w_sb = const.tile([P, TI, D_CONV], f32, name="convw")
    nc.sync.dma_start(conv_w_sb[:], io["conv_w"].rearrange("(t p) k -> p t k", p=P))
    conv_b_sb = const.tile([P, TI], f32, name="convb")
    nc.sync.dma_start(conv_b_sb[:], io["conv_b"].rearrange("(t p) -> p t", p=P))

    u_act, u_bf, silu_z = [], [], []
    for t in range(TI):
        uraw = scanp.tile([P, BL], f32, name=f"uraw{t}")
        nc.vector.tensor_copy(uraw[:], uz_ps[t][:])
        sz = scanp.tile([P, BL], f32, name=f"siluz{t}")
        nc.scalar.activation(sz[:], uz_ps[2 + t][:], Act.Silu)
        silu_z.append(sz)

        acc = scanp.tile([P, BL], f32, name=f"conv{t}")
        for b in range(B):
            o = b * L
            nc.vector.tensor_scalar_mul(
                acc[:, o:o + L], uraw[:, o:o + L], conv_w_sb[:, t, 3:4])
            for k in range(3):
                s = 3 - k
                nc.vector.scalar_tensor_tensor(
                    out=acc[:, o + s:o + L], in0=uraw[:, o:o + L - s],
                    scalar=conv_w_sb[:, t, k:k + 1], in1=acc[:, o + s:o + L],
                    op0=Alu.mult, op1=Alu.add,
                )
        ua = scanp.tile([P, BL], f32, name=f"uact{t}")
        nc.scalar.activation(ua[:], acc[:], Act.Silu, bias=conv_b_sb[:, t:t + 1])
        u_act.append(ua)
        ub = scanp.tile([P, BL], bf16, name=f"ubf{t}")
        nc.vector.tensor_copy(ub[:], ua[:])
        u_bf.append(ub)

    # ---------------- x_proj -> dbc partial -> AllReduce ----------------
    xp_sb = const.tile([P, TI, DT_RANK + 2 * D_STATE], bf16, name="xprojT")
    nc.sync.dma_start(xp_sb[:], io["x_projT"].rearrange("(t p) r -> p t r", p=P))
    dbc_ps = psum.tile([96, BL], f32, name="dbcps", tag="mmps")
    for t in range(TI):
        nc.tensor.matmul(dbc_ps[:], lhsT=xp_sb[:, t, :], rhs=u_bf[t][:],
                         start=(t == 0), stop=(t == TI - 1))
    dbc_par = scanp.tile([96, BL], f32, name="dbcpar")
    nc.vector.tensor_copy(dbc_par[:], dbc_ps[:])
    ar2_in = dram.tile([96, BL], f32, name="ar2_in")
    ar2_out = dram.tile([96, BL], f32, name="ar2_out", addr_space="Shared")
    nc.sync.dma_start(ar2_in[:, :], dbc_par[:])
    nc.gpsimd.collective_compute(
        "AllReduce", Alu.add, replica_groups=RG,
        ins=[ar2_in.opt()], outs=[ar2_out.opt()],
    )

    # ---------------- dt = softplus(dt_projT.T @ dt_r + b) ----------------
    dbc_bf = scanp.tile([P, BL], bf16, name="dbcbf")
    nc.vector.memset(dbc_bf[:], 0.0)
    dbc_f = scanp.tile([96, BL], f32, name="dbcf")
    nc.sync.dma_start(dbc_f[:], ar2_out[:, :])
    nc.vector.tensor_copy(dbc_bf[0:DT_RANK, :], dbc_f[0:DT_RANK, :])

    dtp_sb = const.tile([P, DI_SH], bf16, name="dtprojT")
    nc.sync.dma_start(dtp_sb[:], io["dt_projT"][:, :])
    dtb_sb = const.tile([P, TI], f32, name="dtb")
    nc.sync.dma_start(dtb_sb[:], io["dt_b"].rearrange("(t p) -> p t", p=P))
    A_sb = const.tile([P, TI, D_STATE], f32, name="A")
    nc.sync.dma_start(A_sb[:], io["A"].rearrange("(t p) n -> p t n", p=P))
    D_sb = const.tile([P, TI], f32, name="D")
    nc.sync.dma_start(D_sb[:], io["D"].rearrange("(t p) -> p t", p=P))

    onesf_sb = const.tile([P, 1], f32, name="onesf")
    nc.vector.memset(onesf_sb[:], 1.0)
    dt_t = []
    for t in range(TI):
        ps = psum.tile([P, BL], f32, name=f"dtps{t}", tag="mmps")
        nc.tensor.matmul(ps[:], lhsT=dtp_sb[:, t * P:(t + 1) * P], rhs=dbc_bf[:],
                         start=True, stop=True)
        # softplus(x+b) = log1p(exp(x+b)); args here are well within range
        edt = scanp.tile([P, BL], f32, name=f"edt{t}")
        nc.scalar.activation(edt[:], ps[:], Act.Exp, bias=dtb_sb[:, t:t + 1])
        dtt = scanp.tile([P, BL], f32, name=f"dt{t}")
        nc.scalar.activation(dtt[:], edt[:], Act.Ln, bias=onesf_sb[:])
        dt_t.append(dtt)

    # ---------------- selective scan ----------------
    NL = D_STATE * L
    y_t = [scanp.tile([P, BL], f32, name=f"y{t}") for t in range(TI)]
    for b in range(B):
        Brep = big.tile([P, D_STATE, L], f32, tag="Brep")
        nc.sync.dma_start(
            Brep[:], ar2_out[DT_RANK:DT_RANK + D_STATE,
                             b * L:(b + 1) * L][None, :, :]
            .to_broadcast((P, D_STATE, L)))
        Crep = big.tile([P, D_STATE, L], f32, tag="Crep")
        nc.sync.dma_start(
            Crep[:], ar2_out[DT_RANK + D_STATE:DT_RANK + 2 * D_STATE,
                             b * L:(b + 1) * L][None, :, :]
            .to_broadcast((P, D_STATE, L)))
        for t in range(TI):
            dtb_ = dt_t[t][:, b * L:(b + 1) * L]
            ub_ = u_act[t][:, b * L:(b + 1) * L]

            dA = big.tile([P, NL], f32, tag="dA")
            dAv = dA[:].rearrange("p (n l) -> p n l", n=D_STATE)
            for n in range(D_STATE):
                nc.scalar.activation(dAv[:, n, :], dtb_, Act.Exp,
                                     scale=A_sb[:, t, n:n + 1])
            nc.vector.memset(dAv[:, :, 0:1], 0.0)

            du = scanp.tile([P, L], f32, tag="du")
            nc.vector.tensor_tensor(du[:], dtb_, ub_, Alu.mult)
            dBu = big.tile([P, NL], f32, tag="dBu")
            nc.gpsimd.tensor_tensor(
                dBu[:].rearrange("p (n l) -> p n l", n=D_STATE),
                du[:, None, :].to_broadcast((P, D_STATE, L)),
                Brep[:], Alu.mult)

            h = big.tile([P, NL], f32, tag="h")
            nc.vector.tensor_tensor_scan(h[:], dA[:], dBu[:], 0.0,
                                         Alu.mult, Alu.add)

            hC = big.tile([P, NL], f32, tag="dBu")
            nc.vector.tensor_tensor(hC[:], h[:], Crep[:], Alu.mult)
            yb = y_t[t][:, b * L:(b + 1) * L]
            nc.vector.tensor_reduce(
                yb, hC[:].rearrange("p (n l) -> p l n", n=D_STATE),
                axis=mybir.AxisListType.X, op=Alu.add)
            nc.vector.scalar_tensor_tensor(
                out=yb, in0=ub_, scalar=D_sb[:, t:t + 1], in1=yb,
                op0=Alu.mult, op1=Alu.add)

    # ---------------- gate + out_proj partial -> AllReduce ----------------
    y_bf = []
    for t in range(TI):
        yb16 = scanp.tile([P, BL], bf16, name=f"ybf{t}")
        nc.vector.tensor_tensor(yb16[:], y_t[t][:], silu_z[t][:], Alu.mult)
        y_bf.append(yb16)

    ar3_in = dram.tile([D_MODEL, BL], f32, name="ar3_in")
    ar3_out = dram.tile([D_MODEL, BL], f32, name="ar3_out", addr_space="Shared")
    hy_ps = [psum.tile([P, BL], f32, name=f"hyps{m}", tag="mmps") for m in range(MD)]
    for t in range(TI):
        opk = wstream.tile([P, D_MODEL], bf16, tag="wk")
        nc.sync.dma_start(opk[:, :D_MODEL], io["out_projT"][t * P:(t + 1) * P, :])
        for m in range(MD):
            nc.tensor.matmul(hy_ps[m][:], lhsT=opk[:, m * P:(m + 1) * P],
                             rhs=y_bf[t][:], start=(t == 0), stop=(t == TI - 1))
    for m in range(MD):
        hp = outp.tile([P, BL], f32, tag="hypar")
        nc.vector.tensor_copy(hp[:], hy_ps[m][:])
        nc.sync.dma_start(ar3_in[m * P:(m + 1) * P, :], hp[:])
    nc.gpsimd.collective_compute(
        "AllReduce", Alu.add, replica_groups=RG,
        ins=[ar3_in.opt()], outs=[ar3_out.opt()],
    )

    # ---------------- residual + decoder ----------------
    decb_sb = const.tile([P, MF], f32, name="decb")
    nc.sync.dma_start(decb_sb[:], io["dec_b"].rearrange("(o p) -> p o", p=P))

    h_bf = []
    for m in range(MD):
        hm = latp.tile([P, BL], f32, name=f"hm{m}", tag="xnhm")
        nc.sync.dma_start(hm[:], ar3_out[m * P:(m + 1) * P, :])
        hb = latp.tile([P, BL], bf16, name=f"hbf{m}")
        nc.vector.tensor_tensor(hb[:], hm[:], lat_sb[m][:], Alu.add)
        h_bf.append(hb)

    for m in range(MF):
        dwm = wstream.tile([P, MD, P], bf16, tag="dwm")
        nc.sync.dma_start(dwm[:], io["dec_wT"][m].rearrange("(ko p) f -> p ko f", p=P))
        ps = psum.tile([P, BL], f32, name=f"decps{m % 4}", tag="mmps")
        for k in range(MD):
            nc.tensor.matmul(ps[:], lhsT=dwm[:, k, :], rhs=h_bf[k][:],
                             start=(k == 0), stop=(k == MD - 1))
        ot = outp.tile([P, BL], bf16, tag="ot")
        nc.scalar.activation(ot[:], ps[:], Act.Sigmoid, bias=decb_sb[:, m:m + 1])
        nc.sync.dma_start(io["out"][m * P:(m + 1) * P, :], ot[:])

    ctx.close()


_CACHE = {}


def _get_compiled(repeat=1):
    if ("nc", repeat) in _CACHE:
        return _CACHE[("nc", repeat)]
    nc = bacc.Bacc("TRN2", target_bir_lowering=False, debug=False,
                   num_devices=N_CORES)

    def inp(name, shape, dt=bf16):
        return nc.dram_tensor(name, list(shape), dt, kind="ExternalInput").ap()

    io = {
        "xT": inp("xT", (FLAT_SH, BL)),
        "enc_wT": inp("enc_wT", (FLAT_SH, D_MODEL)),
        "enc_b": inp("enc_b", (D_MODEL,), f32),
        "in_projT": inp("in_projT", (D_MODEL, 2 * DI_SH)),
        "conv_w": inp("conv_w", (DI_SH, D_CONV), f32),
        "conv_b": inp("conv_b", (DI_SH,), f32),
        "x_projT": inp("x_projT", (DI_SH, DT_RANK + 2 * D_STATE)),
        "dt_projT": inp("dt_projT", (P, DI_SH)),
        "dt_b": inp("dt_b", (DI_SH,), f32),
        "A": inp("A", (DI_SH, D_STATE), f32),
        "D": inp("D", (DI_SH,), f32),
        "out_projT": inp("out_projT", (DI_SH, D_MODEL)),
        "dec_wT": inp("dec_wT", (MF, D_MODEL, P)),
        "dec_b": inp("dec_b", (FLAT_SH,), f32),
        "out": nc.dram_tensor("out", [FLAT_SH, BL], bf16,
                              kind="ExternalOutput").ap(),
    }
    with tile.TileContext(nc) as tc:
        for _ in range(repeat):
            _body(tc, io)
    nc.compile()
    _CACHE[("nc", repeat)] = nc
    return nc


def _shard_inputs(x, enc_w, enc_b, dec_w, dec_b, norm_w, in_proj_w, conv_w,
                  conv_b, x_proj_w, dt_proj_w, dt_proj_b, A_log, D_skip,
                  out_proj_w):
    """Host-side preprocessing: transposes, folds, dtype casts, sharding."""
    x2d = np.ascontiguousarray(x.reshape(BL, FLAT).T)          # (FLAT, BL)
    xT = x2d.astype(BF16)
    enc_wT = np.ascontiguousarray(enc_w.T).astype(BF16)        # (FLAT, D_MODEL)
    Wp = (in_proj_w * norm_w[None, :])                         # fold rmsnorm scale
    A = -np.exp(A_log).astype(np.float32)                      # (D_INNER, D_STATE)
    dt_projT = np.ascontiguousarray(dt_proj_w.T)               # (64, D_INNER)
    x_projT = np.ascontiguousarray(x_proj_w.T)                 # (D_INNER, 96)
    out_projT = np.ascontiguousarray(out_proj_w.T)             # (D_INNER, D_MODEL)
    conv_w2 = conv_w.reshape(D_CONV, D_INNER)                  # (4, D_INNER)

    in_maps = []
    for i in range(N_CORES):
        fsl = slice(i * FLAT_SH, (i + 1) * FLAT_SH)
        dsl = slice(i * DI_SH, (i + 1) * DI_SH)
        Wi = np.concatenate([Wp[dsl], Wp[D_INNER + i * DI_SH:
                                         D_INNER + (i + 1) * DI_SH]], axis=0)
        in_projT = np.ascontiguousarray(Wi.T).astype(BF16)     # (D_MODEL, 512)
        dtp = np.zeros((P, DI_SH), np.float32)
        dtp[:DT_RANK] = dt_projT[:, dsl]
        dec_wTi = np.ascontiguousarray(
            dec_w[fsl].reshape(MF, P, D_MODEL).transpose(0, 2, 1)).astype(BF16)
        in_maps.append({
            "xT": np.ascontiguousarray(xT[fsl]),
            "enc_wT": np.ascontiguousarray(enc_wT[fsl]),
            "enc_b": enc_b.astype(np.float32),
            "in_projT": in_projT,
            "conv_w": np.ascontiguousarray(conv_w2[:, dsl].T).astype(np.float32),
            "conv_b": conv_b[dsl].astype(np.float32),
            "x_projT": np.ascontiguousarray(x_projT[dsl]).astype(BF16),
            "dt_projT": dtp.astype(BF16),
            "dt_b": dt_proj_b[dsl].astype(np.float32),
            "A": np.ascontiguousarray(A[dsl]),
            "D": D_skip[dsl].astype(np.float32),
            "out_projT": np.ascontiguousarray(out_projT[dsl]).astype(BF16),
            "dec_wT": dec_wTi,
            "dec_b": dec_b[fsl].astype(np.float32),
        })
    return in_maps


def run(inputs, trace=False, tmpdir=None):
    """Run on hardware; returns (probs, BassKernelResults)."""
    nc = _get_compiled()
    in_maps = _shard_inputs(**{k: np.asarray(v) for k, v in inputs.items()})
    res = bass_utils.run_bass_kernel_spmd(
        nc, in_maps, core_ids=list(range(N_CORES)), trace=trace, tmpdir=tmpdir)
    outT = np.concatenate([res.results[c]["out"] for c in range(N_CORES)],
                          axis=0)                              # (FLAT, BL) bf16
    probs = outT.T.astype(np.float32).reshape(B, L, H, W)[:, :L - 1]
    return probs, res


def kernel(**inputs):
    return run(inputs, trace=False)[0]
